# revision 1
# baseline (speedup 1.0000x reference)
"""Trainium2 Bass kernel for nn_ComplexCrossAttention.

Sharding: 8 cores = 2 batches x 4 head-groups (4 heads each).
Each core computes, for its (b, head-group):
  - complex Q/K/V projections (column-sharded by head) in transposed layout
  - attention scoresT = (qr.kr + qi.ki)*scale with s on partitions
  - softmax (no max-subtraction; scores are provably small) via exp + column-sum
  - av in transposed layout -> OT [d2, l]
  - partial output projection (row-sharded by head)
Host sums the 4 partial y per batch and adds the bias.

All matmuls are N=512 full-rate. Activations enter as bf16 (DMA-xbar
transpose requires 2-byte dtype) via gpsimd cast-DMAs; the scores path
uses float32r (TF32-like, full rate at N>=256).
"""

import sys

import numpy as np

try:
    import concourse.bacc as bacc
except ImportError:  # pragma: no cover - fallback for bare environments
    sys.path.insert(0, "/opt/trn_rl_repo")
    import concourse.bacc as bacc

import concourse.mybir as mybir
import concourse.tile as tile
from concourse.bass_utils import run_bass_kernel_spmd

F32 = mybir.dt.float32
BF16 = mybir.dt.bfloat16
F32R = mybir.dt.float32r

# ---- problem constants (hardcoded per contract) ----
B, L, S, C = 2, 2048, 2048, 1024
H, D = 16, 64
SCALE = float(1.0 / np.sqrt(np.float32(D)))
HPC = 4          # heads per core
D2 = 2 * D       # stacked (real|imag) head dim = 128
NCK = C // 128   # contraction chunks = 8
NLB = L // 512   # l-blocks = 4
NST = S // 128   # s-tiles = 16
NLT = L // 128   # l-tiles = 16
NEB = 2          # e-blocks of 512 in C

# ---- dtype configuration ----
QS_DT = F32R     # Qs/Ks (scores operands)
EXP_DT = BF16    # expT / Vs / ones (av + denom operands)
VS_DT = EXP_DT
OT_DT = BF16     # OT / wo (o-proj operands)

_CACHE = {}


def _build_program():
    nc = bacc.Bacc("TRN2", target_bir_lowering=False, debug=False, num_devices=8)

    # per-core external inputs
    x_r = nc.dram_tensor("x_r", [L, C], F32, kind="ExternalInput")
    x_i = nc.dram_tensor("x_i", [L, C], F32, kind="ExternalInput")
    c_r = nc.dram_tensor("c_r", [S, C], F32, kind="ExternalInput")
    c_i = nc.dram_tensor("c_i", [S, C], F32, kind="ExternalInput")
    # stacked complex projection weights (host-prepared, bf16)
    # wq/wk: [C, HPC, 2, 128]  (c, head, pm, m) ; lhsT tiles
    wq = nc.dram_tensor("wq", [C, HPC, 2, D2], BF16, kind="ExternalInput")
    wk = nc.dram_tensor("wk", [C, HPC, 2, D2], BF16, kind="ExternalInput")
    # wv: [C, 2, HPC*128]  (c, pm, all-head d2) ; rhs tiles
    wv = nc.dram_tensor("wv", [C, 2, HPC * D2], BF16, kind="ExternalInput")
    # wo: [HPC, 128, 2, NEB, 512]  (head, d2row, ri, eblock, e) ; rhs tiles
    wo = nc.dram_tensor("wo", [HPC, D2, 2, NEB, 512], OT_DT, kind="ExternalInput")

    y_r = nc.dram_tensor("y_r", [L, C], F32, kind="ExternalOutput")
    y_i = nc.dram_tensor("y_i", [L, C], F32, kind="ExternalOutput")

    with tile.TileContext(nc) as tc:
        _emit(nc, tc, x_r, x_i, c_r, c_i, wq, wk, wv, wo, y_r, y_i)

    nc.compile()
    return nc


def _emit(nc, tc, x_r, x_i, c_r, c_i, wq, wk, wv, wo, y_r, y_i):
    from contextlib import ExitStack

    ctx = ExitStack()
    with ctx:
        singles = ctx.enter_context(tc.tile_pool(name="singles", bufs=1))
        attn_sb = ctx.enter_context(tc.tile_pool(name="attn_sb", bufs=1))
        dram = ctx.enter_context(tc.tile_pool(name="dram", bufs=1, space="DRAM"))

        # bf16 mirrors of the activations (cast-DMA'd, then xbar-transposed)
        xbf = [dram.tile([L, C], BF16, tag=f"xbf{t}", name=f"xbf{t}") for t in range(2)]
        cbf = [dram.tile([S, C], BF16, tag=f"cbf{t}", name=f"cbf{t}") for t in range(2)]

        # persistent attention operands
        qs = attn_sb.tile([128, HPC, L], QS_DT)            # [d2, h, l]
        ks = attn_sb.tile([128, HPC, S], QS_DT)            # [d2, h, s]
        vs = attn_sb.tile([128, NST, HPC * D2], VS_DT)     # [s-part, st, d2all]

        # ---------- P0a: cast x fp32 -> bf16 (DRAM->DRAM compute DMA) ----------
        # column-halves so transposes of early chunks can start sooner
        for asrc, adst in ((x_r, xbf[0]), (x_i, xbf[1]),
                           (c_r, cbf[0]), (c_i, cbf[1])):
            for cb in range(4):
                csl = slice(cb * 256, (cb + 1) * 256)
                nc.gpsimd.dma_start(out=adst[:, csl], in_=asrc[:, csl])

        # ---------- P1+P2: transpose-in x, Q projection ----------
        with (
            tc.tile_pool(name="xt", bufs=1) as xt_pool,
            tc.tile_pool(name="wqk", bufs=1) as wqk_pool,
            tc.tile_pool(name="ps_proj", bufs=1, space="PSUM") as ps_proj,
        ):
            wq_sb = wqk_pool.tile([128, NCK, HPC, 2, D2], BF16, tag="wq")
            nc.sync.dma_start(
                out=wq_sb, in_=wq.rearrange("(ck p) h pm m -> p ck h pm m", p=128)
            )
            xt = [xt_pool.tile([128, NCK, L], BF16, tag=f"xt{t}", name=f"xt{t}") for t in range(2)]
            for ck in range(NCK):
                for t in range(2):
                    nc.sync.dma_start(
                        out=xt[t][:, ck, :],
                        in_=xbf[t][:, ck * 128:(ck + 1) * 128],
                        transpose=True,
                    )
            for hp in range(HPC // 2):
                pq = [
                    [ps_proj.tile([128, 512], F32, tag=f"pq{hh}{lb}", name=f"pq{hh}{lb}")
                     for lb in range(NLB)]
                    for hh in range(2)
                ]
                n = 2 * NCK
                i = 0
                for ck in range(NCK):
                    for pm in range(2):
                        for hh in range(2):
                            for lb in range(NLB):
                                nc.tensor.matmul(
                                    pq[hh][lb],
                                    wq_sb[:, ck, 2 * hp + hh, pm, :],
                                    xt[pm][:, ck, lb * 512:(lb + 1) * 512],
                                    start=(i == 0),
                                    stop=(i == n - 1),
                                )
                        i += 1
                for hh in range(2):
                    for lb in range(NLB):
                        nc.vector.tensor_copy(
                            out=qs[:, 2 * hp + hh, lb * 512:(lb + 1) * 512], in_=pq[hh][lb]
                        )

        # ---------- P3: transpose ctx, K and V projections ----------
        # score/exp pools open across P3 so the scheduler can hoist
        # scoresT+exp of early heads into K/V-phase gaps.
        exp_pool = ctx.enter_context(tc.tile_pool(name="exp", bufs=2))
        ps_s = ctx.enter_context(tc.tile_pool(name="ps_s", bufs=2, space="PSUM"))

        with (
            tc.tile_pool(name="ct", bufs=1) as ct_pool,
            tc.tile_pool(name="wkv", bufs=1) as wkv_pool,
            tc.tile_pool(name="ps_proj2", bufs=1, space="PSUM") as ps_proj,
            tc.tile_pool(name="ps_v", bufs=2, space="PSUM") as ps_v,
        ):
            wk_sb = wkv_pool.tile([128, NCK, HPC, 2, D2], BF16, tag="wkv")
            nc.sync.dma_start(
                out=wk_sb, in_=wk.rearrange("(ck p) h pm m -> p ck h pm m", p=128)
            )
            ct = [ct_pool.tile([128, NCK, S], BF16, tag=f"ct{t}", name=f"ct{t}") for t in range(2)]
            for ck in range(NCK):
                for t in range(2):
                    nc.sync.dma_start(
                        out=ct[t][:, ck, :],
                        in_=cbf[t][:, ck * 128:(ck + 1) * 128],
                        transpose=True,
                    )
            for h in range(HPC):
                for rnd in range(2):
                    pk = [ps_proj.tile([128, 512], F32, tag=f"pk{j}", name=f"pk{j}") for j in range(2)]
                    n = 2 * NCK
                    i = 0
                    for ck in range(NCK):
                        for pm in range(2):
                            for j in range(2):
                                sb = 2 * rnd + j
                                nc.tensor.matmul(
                                    pk[j],
                                    wk_sb[:, ck, h, pm, :],
                                    ct[pm][:, ck, sb * 512:(sb + 1) * 512],
                                    start=(i == 0),
                                    stop=(i == n - 1),
                                )
                            i += 1
                    for j in range(2):
                        sb = 2 * rnd + j
                        nc.vector.tensor_copy(out=ks[:, h, sb * 512:(sb + 1) * 512], in_=pk[j])
            wv_sb = wkv_pool.tile([128, NCK, 2, HPC * D2], BF16, tag="wkv", name="wv_sb")
            nc.sync.dma_start(
                out=wv_sb, in_=wv.rearrange("(ck p) pm n -> p ck pm n", p=128)
            )
            for st in range(NST):
                pv = ps_v.tile([128, 512], F32, tag="pv")
                n = 2 * NCK
                i = 0
                for ck in range(NCK):
                    for pm in range(2):
                        nc.tensor.matmul(
                            pv,
                            ct[pm][:, ck, st * 128:(st + 1) * 128],
                            wv_sb[:, ck, pm, :],
                            start=(i == 0),
                            stop=(i == n - 1),
                        )
                        i += 1
                nc.vector.tensor_copy(out=vs[:, st, :], in_=pv)

        # ---------- P4+P5 fused: attention + output projection, lb-outer ----------
        with (
            tc.tile_pool(name="late", bufs=1) as late_pool,
            tc.tile_pool(name="exp2", bufs=3) as exp_pool2,
            tc.tile_pool(name="otp", bufs=2) as ot_pool,
            tc.tile_pool(name="ysb", bufs=4) as ysb_pool,
            tc.tile_pool(name="ps_d", bufs=1, space="PSUM") as ps_d,
            tc.tile_pool(name="ps_o", bufs=1, space="PSUM") as ps_o,
            tc.tile_pool(name="ps_y", bufs=1, space="PSUM") as ps_y,
        ):
            ones = late_pool.tile([128, D2], EXP_DT)
            nc.vector.memset(ones, 1.0)
            wo_sb = late_pool.tile([128, HPC, 2, NEB, 512], OT_DT, tag="wo", name="wo_sb")
            nc.sync.dma_start(out=wo_sb, in_=wo.rearrange("h p ri eb e -> p h ri eb e"))
            for lb in range(NLB):
                lsl = slice(lb * 512, (lb + 1) * 512)
                ot = ot_pool.tile([128, HPC, 512], OT_DT, tag="ot", name="ot")
                for h in range(HPC):
                    pool_h = exp_pool if lb == 0 else exp_pool2
                    expt = pool_h.tile([128, NST, 512], EXP_DT, tag="expt", name="expt")
                    for pr in range(NST // 2):
                        pscore = ps_s.tile([128, 2, 512], F32, tag="pscore")
                        for j in range(2):
                            st = 2 * pr + j
                            nc.tensor.matmul(
                                pscore[:, j, :],
                                ks[:, h, st * 128:(st + 1) * 128],
                                qs[:, h, lsl],
                                start=True,
                                stop=True,
                                skip_group_check=True,
                            )
                        nc.scalar.activation(
                            out=expt[:, 2 * pr:2 * pr + 2, :],
                            in_=pscore,
                            func=mybir.ActivationFunctionType.Exp,
                            scale=SCALE,
                        )
                    # av: OT[d2, l] accumulated over s-tiles (reads expt first)
                    pav = ps_o.tile([128, 512], F32, tag="pav")
                    for st in range(NST):
                        nc.tensor.matmul(
                            pav,
                            vs[:, st, h * D2:(h + 1) * D2],
                            expt[:, st, :],
                            start=(st == 0),
                            stop=(st == NST - 1),
                        )
                    # in-place pairwise tree-sum of the 16 s-tiles (WAR after av)
                    for step in (1, 2, 4, 8):
                        eng = nc.gpsimd if step == 1 else nc.vector
                        for j in range(0, NST, 2 * step):
                            eng.tensor_add(
                                out=expt[:, j, :], in0=expt[:, j, :], in1=expt[:, j + step, :]
                            )
                    pden = ps_d.tile([128, 512], F32, tag="pden")
                    nc.tensor.matmul(pden, ones, expt[:, 0, :], start=True, stop=True)
                    recip = ot_pool.tile([128, 512], F32, tag="recip")
                    nc.vector.reciprocal(out=recip, in_=pden)
                    nc.vector.tensor_mul(out=ot[:, h, :], in0=pav, in1=recip)

                # output projection for this l-block (needs all heads' ot)
                for jt in range(4):
                    lt = lb * 4 + jt
                    lrow = slice(lt * 128, (lt + 1) * 128)
                    for eb in range(NEB):
                        esl = slice(eb * 512, (eb + 1) * 512)
                        py = [ps_y.tile([128, 512], F32, tag=f"py{ri}", name=f"py{ri}")
                              for ri in range(2)]
                        for h in range(HPC):
                            for ri in range(2):
                                nc.tensor.matmul(
                                    py[ri],
                                    ot[:, h, jt * 128:(jt + 1) * 128],
                                    wo_sb[:, h, ri, eb, :],
                                    start=(h == 0),
                                    stop=(h == HPC - 1),
                                )
                        yr_t = ysb_pool.tile([128, 512], F32, tag="yrt")
                        nc.vector.tensor_copy(out=yr_t, in_=py[0])
                        nc.sync.dma_start(out=y_r[lrow, esl], in_=yr_t)
                        yi_t = ysb_pool.tile([128, 512], F32, tag="yit")
                        nc.vector.tensor_copy(out=yi_t, in_=py[1])
                        nc.sync.dma_start(out=y_i[lrow, esl], in_=yi_t)


def _prep_core_inputs(inputs, core):
    """Slice + host-prepare the weight layouts for one core."""
    import ml_dtypes

    b = core // 4
    g = core % 4
    hcols = slice(g * HPC * D, (g + 1) * HPC * D)  # 256 channel cols/rows

    wq_r = inputs["wq_r"][:, hcols]
    wq_i = inputs["wq_i"][:, hcols]
    wk_r = inputs["wk_r"][:, hcols]
    wk_i = inputs["wk_i"][:, hcols]
    wv_r = inputs["wv_r"][:, hcols]
    wv_i = inputs["wv_i"][:, hcols]
    wo_r = inputs["wo_r"][hcols, :]
    wo_i = inputs["wo_i"][hcols, :]

    def stack_lhst(wr, wi):
        # [C, HPC, 2, D2]: pm=0 -> [wr | wi], pm=1 -> [-wi | wr]
        out = np.empty((C, HPC, 2, D2), np.float32)
        for hh in range(HPC):
            cs = slice(hh * D, (hh + 1) * D)
            out[:, hh, 0, :D] = wr[:, cs]
            out[:, hh, 0, D:] = wi[:, cs]
            out[:, hh, 1, :D] = -wi[:, cs]
            out[:, hh, 1, D:] = wr[:, cs]
        return out.astype(ml_dtypes.bfloat16)

    def stack_rhs_v(wr, wi):
        # [C, 2, HPC*D2]
        out = np.empty((C, 2, HPC * D2), np.float32)
        for hh in range(HPC):
            cs = slice(hh * D, (hh + 1) * D)
            out[:, 0, hh * D2:hh * D2 + D] = wr[:, cs]
            out[:, 0, hh * D2 + D:(hh + 1) * D2] = wi[:, cs]
            out[:, 1, hh * D2:hh * D2 + D] = -wi[:, cs]
            out[:, 1, hh * D2 + D:(hh + 1) * D2] = wr[:, cs]
        return out.astype(ml_dtypes.bfloat16)

    def stack_wo(wr, wi):
        # [HPC, D2, 2, NEB, 512]; rows 0:D multiply Or, D:D2 multiply Oi
        out = np.empty((HPC, D2, 2, NEB, 512), np.float32)
        for hh in range(HPC):
            rs = slice(hh * D, (hh + 1) * D)
            for eb in range(NEB):
                esl = slice(eb * 512, (eb + 1) * 512)
                out[hh, :D, 0, eb, :] = wr[rs, esl]
                out[hh, D:, 0, eb, :] = -wi[rs, esl]
                out[hh, :D, 1, eb, :] = wi[rs, esl]
                out[hh, D:, 1, eb, :] = wr[rs, esl]
        return out.astype(ml_dtypes.bfloat16)

    return {
        "x_r": np.ascontiguousarray(inputs["inputs_real"][b]),
        "x_i": np.ascontiguousarray(inputs["inputs_imag"][b]),
        "c_r": np.ascontiguousarray(inputs["context_real"][b]),
        "c_i": np.ascontiguousarray(inputs["context_imag"][b]),
        "wq": stack_lhst(wq_r, wq_i),
        "wk": stack_lhst(wk_r, wk_i),
        "wv": stack_rhs_v(wv_r, wv_i),
        "wo": stack_wo(wo_r, wo_i),
    }


def get_program():
    if "nc" not in _CACHE:
        _CACHE["nc"] = _build_program()
    return _CACHE["nc"]


def kernel(**inputs):
    nc = get_program()
    in_maps = [_prep_core_inputs(inputs, core) for core in range(8)]
    res = run_bass_kernel_spmd(nc, in_maps, core_ids=list(range(8)))

    yr = np.zeros((B, L, C), np.float32)
    yi = np.zeros((B, L, C), np.float32)
    for core in range(8):
        b = core // 4
        yr[b] += res.results[core]["y_r"]
        yi[b] += res.results[core]["y_i"]
    yr += inputs["bo_r"][None, None, :]
    yi += inputs["bo_i"][None, None, :]
    return np.stack([yr, yi], axis=0)



# revision 4
# speedup vs baseline: 1.3463x; 1.3463x over previous
"""Trainium2 Bass kernel for nn_ComplexCrossAttention.

Sharding: 8 cores = 2 batches x 4 head-groups (4 heads each).

Host prep (free for the HW metric): activations are transposed to [C, L]
and cast to bf16 on the host, so the kernel needs no DMA-xbar transposes
and no fp32->bf16 cast DMAs. Weights are pre-stacked for the complex
matmuls.

Per-core program:
  Phase KV (per s-block of 512): K projection (stacked complex lhsT) and
    V projection share streamed ct chunks; ct lands via the SP HWDGE
    queue while xt/weights land via the Activation HWDGE queue.
  Phase Q: stacked projection from resident xt.
  Phase ATTN (per (l-block, head)): scoresT = (qr.kr+qi.ki), exp via
    scalar activation (scale folded in), av in transposed layout,
    denominator via ones-matmul of tree-summed exp tiles, then output
    projection per l-block. y partials summed on host across head-groups.
"""

import sys

import numpy as np

try:
    import concourse.bacc as bacc
except ImportError:  # pragma: no cover - fallback for bare environments
    sys.path.insert(0, "/opt/trn_rl_repo")
    import concourse.bacc as bacc

import concourse.mybir as mybir
import concourse.tile as tile
from concourse.bass_utils import run_bass_kernel_spmd

F32 = mybir.dt.float32
BF16 = mybir.dt.bfloat16

# ---- problem constants (hardcoded per contract) ----
B, L, S, C = 2, 2048, 2048, 1024
H, D = 16, 64
SCALE = float(1.0 / np.sqrt(np.float32(D)))
HPC = 4          # heads per core
D2 = 2 * D       # stacked (real|imag) head dim = 128
NCK = C // 128   # contraction chunks = 8
NLB = L // 512   # l-blocks = 4
NSB = S // 512   # s-blocks = 4
NST = S // 128   # s-tiles = 16
NEB = 2          # e-blocks of 512 in C

_CACHE = {}


def _build_program():
    nc = bacc.Bacc("TRN2", target_bir_lowering=False, debug=False, num_devices=8)

    # per-core external inputs (host pre-transposed/cast/stacked)
    xt_r = nc.dram_tensor("xt_r", [C, L], BF16, kind="ExternalInput")
    xt_i = nc.dram_tensor("xt_i", [C, L], BF16, kind="ExternalInput")
    ct_r = nc.dram_tensor("ct_r", [C, S], BF16, kind="ExternalInput")
    ct_i = nc.dram_tensor("ct_i", [C, S], BF16, kind="ExternalInput")
    # wq/wk: [C, HPC, 2, D2]  (c, head, pm, m) ; lhsT tiles
    wq = nc.dram_tensor("wq", [C, HPC, 2, D2], BF16, kind="ExternalInput")
    wk = nc.dram_tensor("wk", [C, HPC, 2, D2], BF16, kind="ExternalInput")
    # wv: [C, 2, HPC*128]  (c, pm, all-head d2) ; rhs tiles
    wv = nc.dram_tensor("wv", [C, 2, HPC * D2], BF16, kind="ExternalInput")
    # wo: [HPC, 128, 2, NEB, 512]  (head, d2row, ri, eblock, e) ; rhs tiles
    wo = nc.dram_tensor("wo", [HPC, D2, 2, NEB, 512], BF16, kind="ExternalInput")

    y_r = nc.dram_tensor("y_r", [L, C], F32, kind="ExternalOutput")
    y_i = nc.dram_tensor("y_i", [L, C], F32, kind="ExternalOutput")

    with tile.TileContext(nc) as tc:
        _emit(nc, tc, xt_r, xt_i, ct_r, ct_i, wq, wk, wv, wo, y_r, y_i)

    nc.compile()
    return nc


def _emit(nc, tc, xt_r, xt_i, ct_r, ct_i, wq, wk, wv, wo, y_r, y_i):
    from contextlib import ExitStack

    ctx = ExitStack()
    with ctx:
        persist = ctx.enter_context(tc.tile_pool(name="persist", bufs=1))

        # persistent attention operands (all bf16)
        qs = persist.tile([128, HPC, L], BF16)            # [d2, h, l]
        ks = persist.tile([128, HPC, S], BF16)            # [d2, h, s]
        vs = persist.tile([128, NST, HPC * D2], BF16)     # [s-part, st, d2all]

        # ---------- Phase KV: K + V projections, streaming ct by s-block ----
        # xt/wq live through phase Q, then free before the attention pools.
        with tc.tile_pool(name="xtw", bufs=1) as xtw_pool:
            with (
                tc.tile_pool(name="ctc", bufs=2) as ctc_pool,
                tc.tile_pool(name="wkv", bufs=1) as wkv_pool,
                tc.tile_pool(name="ps_k", bufs=2, space="PSUM") as ps_k,
                tc.tile_pool(name="ps_v", bufs=2, space="PSUM") as ps_v,
            ):
                # K/V weights go out first on the Activation HWDGE queue so
                # the first K matmul only waits on them + the first ct chunk.
                wk_sb = wkv_pool.tile([128, NCK, HPC, 2, D2], BF16, tag="wk", name="wk_sb")
                nc.scalar.dma_start(
                    out=wk_sb, in_=wk.rearrange("(ck p) h pm m -> p ck h pm m", p=128)
                )
                wv_sb = wkv_pool.tile([128, NCK, 2, HPC * D2], BF16, tag="wv", name="wv_sb")
                nc.scalar.dma_start(
                    out=wv_sb, in_=wv.rearrange("(ck p) pm n -> p ck pm n", p=128)
                )
                # ct streams on the SP HWDGE queue, one chunk per s-block
                ctcs = []
                for sb in range(NSB):
                    ssl = slice(sb * 512, (sb + 1) * 512)
                    ctc = ctc_pool.tile([128, NCK, 2, 512], BF16, tag="ctc", name="ctc")
                    nch = 2 if sb == 0 else 1
                    for t, src in ((0, ct_r), (1, ct_i)):
                        for ch in range(nch):
                            cs = slice(ch * NCK // nch, (ch + 1) * NCK // nch)
                            nc.sync.dma_start(
                                out=ctc[:, cs, t, :],
                                in_=src.rearrange("(ck p) s -> p ck s", p=128)[:, cs, ssl],
                            )
                    ctcs.append(ctc)
                # xt + Q weights stream behind wk/wv on the Activation queue
                xt = xtw_pool.tile([128, NCK, 2, L], BF16, tag="xt", name="xt")
                for t, src in ((0, xt_r), (1, xt_i)):
                    nc.scalar.dma_start(
                        out=xt[:, :, t, :],
                        in_=src.rearrange("(ck p) l -> p ck l", p=128),
                    )
                wq_sb = xtw_pool.tile([128, NCK, HPC, 2, D2], BF16, tag="wq", name="wq_sb")
                nc.scalar.dma_start(
                    out=wq_sb, in_=wq.rearrange("(ck p) h pm m -> p ck h pm m", p=128)
                )

                for sb in range(NSB):
                    ssl = slice(sb * 512, (sb + 1) * 512)
                    ctc = ctcs[sb]
                    for hp in range(HPC // 2):
                        pk = ps_k.tile([128, 2, 512], F32, tag="pk", name="pk")
                        n = 2 * NCK
                        i = 0
                        for ck in range(NCK):
                            for pm in range(2):
                                for hh in range(2):
                                    nc.tensor.matmul(
                                        pk[:, hh, :],
                                        wk_sb[:, ck, 2 * hp + hh, pm, :],
                                        ctc[:, ck, pm, :],
                                        start=(i == 0),
                                        stop=(i == n - 1),
                                        skip_group_check=True,
                                    )
                                i += 1
                        for hh in range(2):
                            nc.vector.tensor_copy(
                                out=ks[:, 2 * hp + hh, ssl], in_=pk[:, hh, :]
                            )
                    for jt in range(4):
                        st = sb * 4 + jt
                        pv = ps_v.tile([128, 512], F32, tag="pv", name="pv")
                        n = 2 * NCK
                        i = 0
                        for ck in range(NCK):
                            for pm in range(2):
                                nc.tensor.matmul(
                                    pv,
                                    ctc[:, ck, pm, jt * 128:(jt + 1) * 128],
                                    wv_sb[:, ck, pm, :],
                                    start=(i == 0),
                                    stop=(i == n - 1),
                                )
                                i += 1
                        nc.vector.tensor_copy(out=vs[:, st, :], in_=pv)

            # ---------- Phase Q: Q projection from resident xt ----------
            with tc.tile_pool(name="ps_q", bufs=2, space="PSUM") as ps_q:
                for lb in range(NLB):
                    lsl = slice(lb * 512, (lb + 1) * 512)
                    for hp in range(HPC // 2):
                        pq = ps_q.tile([128, 2, 512], F32, tag="pq", name="pq")
                        n = 2 * NCK
                        i = 0
                        for ck in range(NCK):
                            for pm in range(2):
                                for hh in range(2):
                                    nc.tensor.matmul(
                                        pq[:, hh, :],
                                        wq_sb[:, ck, 2 * hp + hh, pm, :],
                                        xt[:, ck, pm, lsl],
                                        start=(i == 0),
                                        stop=(i == n - 1),
                                        skip_group_check=True,
                                    )
                                i += 1
                        for hh in range(2):
                            nc.vector.tensor_copy(
                                out=qs[:, 2 * hp + hh, lsl], in_=pq[:, hh, :]
                            )

        # ---------- Phase ATTN: attention + output projection, lb-outer ----
        with (
            tc.tile_pool(name="late", bufs=1) as late_pool,
            tc.tile_pool(name="expp", bufs=3) as exp_pool,
            tc.tile_pool(name="otp", bufs=2) as ot_pool,
            tc.tile_pool(name="ysb", bufs=4) as ysb_pool,
            tc.tile_pool(name="ps_s", bufs=2, space="PSUM") as ps_s,
            tc.tile_pool(name="ps_o", bufs=1, space="PSUM") as ps_o,
            tc.tile_pool(name="ps_d", bufs=1, space="PSUM") as ps_d,
            tc.tile_pool(name="ps_y", bufs=1, space="PSUM") as ps_y,
        ):
            ones = late_pool.tile([128, D2], BF16)
            nc.vector.memset(ones, 1.0)
            wo_sb = late_pool.tile([128, HPC, 2, NEB, 512], BF16, tag="wo", name="wo_sb")
            nc.scalar.dma_start(out=wo_sb, in_=wo.rearrange("h p ri eb e -> p h ri eb e"))
            for lb in range(NLB):
                lsl = slice(lb * 512, (lb + 1) * 512)
                ot = ot_pool.tile([128, HPC, 512], BF16, tag="ot", name="ot")
                for h in range(HPC):
                    expt = exp_pool.tile([128, NST, 512], BF16, tag="expt", name="expt")
                    for pr in range(NST // 2):
                        pscore = ps_s.tile([128, 2, 512], F32, tag="pscore", name="pscore")
                        for j in range(2):
                            st = 2 * pr + j
                            nc.tensor.matmul(
                                pscore[:, j, :],
                                ks[:, h, st * 128:(st + 1) * 128],
                                qs[:, h, lsl],
                                start=True,
                                stop=True,
                                skip_group_check=True,
                            )
                        nc.scalar.activation(
                            out=expt[:, 2 * pr:2 * pr + 2, :],
                            in_=pscore,
                            func=mybir.ActivationFunctionType.Exp,
                            scale=SCALE,
                        )
                    # av: OT[d2, l] accumulated over s-tiles (reads expt first)
                    pav = ps_o.tile([128, 512], F32, tag="pav", name="pav")
                    for st in range(NST):
                        nc.tensor.matmul(
                            pav,
                            vs[:, st, h * D2:(h + 1) * D2],
                            expt[:, st, :],
                            start=(st == 0),
                            stop=(st == NST - 1),
                        )
                    # in-place pairwise tree-sum of the 16 s-tiles (WAR after av)
                    for step in (1, 2, 4, 8):
                        eng = nc.gpsimd if step == 1 else nc.vector
                        for j in range(0, NST, 2 * step):
                            eng.tensor_add(
                                out=expt[:, j, :], in0=expt[:, j, :], in1=expt[:, j + step, :]
                            )
                    pden = ps_d.tile([128, 512], F32, tag="pden", name="pden")
                    nc.tensor.matmul(pden, ones, expt[:, 0, :], start=True, stop=True)
                    recip = ot_pool.tile([128, 512], F32, tag="recip", name="recip")
                    nc.vector.reciprocal(out=recip, in_=pden)
                    nc.vector.tensor_mul(out=ot[:, h, :], in0=pav, in1=recip)

                # output projection for this l-block (needs all heads' ot)
                for jt in range(4):
                    lt = lb * 4 + jt
                    lrow = slice(lt * 128, (lt + 1) * 128)
                    for eb in range(NEB):
                        esl = slice(eb * 512, (eb + 1) * 512)
                        py = ps_y.tile([128, 2, 512], F32, tag="py", name="py")
                        for h in range(HPC):
                            for ri in range(2):
                                nc.tensor.matmul(
                                    py[:, ri, :],
                                    ot[:, h, jt * 128:(jt + 1) * 128],
                                    wo_sb[:, h, ri, eb, :],
                                    start=(h == 0),
                                    stop=(h == HPC - 1),
                                    skip_group_check=True,
                                )
                        yr_t = ysb_pool.tile([128, 512], F32, tag="yrt", name="yrt")
                        nc.vector.tensor_copy(out=yr_t, in_=py[:, 0, :])
                        nc.sync.dma_start(out=y_r[lrow, esl], in_=yr_t)
                        yi_t = ysb_pool.tile([128, 512], F32, tag="yit", name="yit")
                        nc.vector.tensor_copy(out=yi_t, in_=py[:, 1, :])
                        nc.sync.dma_start(out=y_i[lrow, esl], in_=yi_t)


def _prep_core_inputs(inputs, core):
    """Slice + host-prepare activations/weights for one core."""
    import ml_dtypes

    b = core // 4
    g = core % 4
    hcols = slice(g * HPC * D, (g + 1) * HPC * D)  # 256 channel cols/rows

    wq_r = inputs["wq_r"][:, hcols]
    wq_i = inputs["wq_i"][:, hcols]
    wk_r = inputs["wk_r"][:, hcols]
    wk_i = inputs["wk_i"][:, hcols]
    wv_r = inputs["wv_r"][:, hcols]
    wv_i = inputs["wv_i"][:, hcols]
    wo_r = inputs["wo_r"][hcols, :]
    wo_i = inputs["wo_i"][hcols, :]

    def stack_lhst(wr, wi):
        # [C, HPC, 2, D2]: pm=0 -> [wr | wi], pm=1 -> [-wi | wr]
        out = np.empty((C, HPC, 2, D2), np.float32)
        for hh in range(HPC):
            cs = slice(hh * D, (hh + 1) * D)
            out[:, hh, 0, :D] = wr[:, cs]
            out[:, hh, 0, D:] = wi[:, cs]
            out[:, hh, 1, :D] = -wi[:, cs]
            out[:, hh, 1, D:] = wr[:, cs]
        return out.astype(ml_dtypes.bfloat16)

    def stack_rhs_v(wr, wi):
        # [C, 2, HPC*D2]
        out = np.empty((C, 2, HPC * D2), np.float32)
        for hh in range(HPC):
            cs = slice(hh * D, (hh + 1) * D)
            out[:, 0, hh * D2:hh * D2 + D] = wr[:, cs]
            out[:, 0, hh * D2 + D:(hh + 1) * D2] = wi[:, cs]
            out[:, 1, hh * D2:hh * D2 + D] = -wi[:, cs]
            out[:, 1, hh * D2 + D:(hh + 1) * D2] = wr[:, cs]
        return out.astype(ml_dtypes.bfloat16)

    def stack_wo(wr, wi):
        # [HPC, D2, 2, NEB, 512]; rows 0:D multiply Or, D:D2 multiply Oi
        out = np.empty((HPC, D2, 2, NEB, 512), np.float32)
        for hh in range(HPC):
            rs = slice(hh * D, (hh + 1) * D)
            for eb in range(NEB):
                esl = slice(eb * 512, (eb + 1) * 512)
                out[hh, :D, 0, eb, :] = wr[rs, esl]
                out[hh, D:, 0, eb, :] = -wi[rs, esl]
                out[hh, :D, 1, eb, :] = wi[rs, esl]
                out[hh, D:, 1, eb, :] = wr[rs, esl]
        return out.astype(ml_dtypes.bfloat16)

    bf = ml_dtypes.bfloat16
    return {
        "xt_r": np.ascontiguousarray(inputs["inputs_real"][b].T).astype(bf),
        "xt_i": np.ascontiguousarray(inputs["inputs_imag"][b].T).astype(bf),
        "ct_r": np.ascontiguousarray(inputs["context_real"][b].T).astype(bf),
        "ct_i": np.ascontiguousarray(inputs["context_imag"][b].T).astype(bf),
        "wq": stack_lhst(wq_r, wq_i),
        "wk": stack_lhst(wk_r, wk_i),
        "wv": stack_rhs_v(wv_r, wv_i),
        "wo": stack_wo(wo_r, wo_i),
    }


def get_program():
    if "nc" not in _CACHE:
        _CACHE["nc"] = _build_program()
    return _CACHE["nc"]


def kernel(**inputs):
    nc = get_program()
    in_maps = [_prep_core_inputs(inputs, core) for core in range(8)]
    res = run_bass_kernel_spmd(nc, in_maps, core_ids=list(range(8)))

    yr = np.zeros((B, L, C), np.float32)
    yi = np.zeros((B, L, C), np.float32)
    for core in range(8):
        b = core // 4
        yr[b] += res.results[core]["y_r"]
        yi[b] += res.results[core]["y_i"]
    yr += inputs["bo_r"][None, None, :]
    yi += inputs["bo_i"][None, None, :]
    return np.stack([yr, yi], axis=0)


# revision 5
# speedup vs baseline: 1.3875x; 1.0306x over previous
"""Trainium2 Bass kernel for nn_ComplexCrossAttention.

Sharding: 8 cores = 2 batches x 4 head-groups (4 heads each).

Host prep (free for the HW metric): activations are transposed to [C, L]
and cast to bf16 on the host, so the kernel needs no DMA-xbar transposes
and no fp32->bf16 cast DMAs. Weights are pre-stacked for the complex
matmuls.

Per-core program (phases ordered to eliminate PE stalls):
  Phase Q  (per l-block): stacked complex Q projection from xt chunks
    streamed on the Activation HWDGE queue.
  Phase KV (per s-block): K and V projections sharing ct chunks streamed
    on the SP HWDGE queue (prefetched during Q).
  Phase ATTN (per (l-block, head)): scoresT = (qr.kr+qi.ki), exp via
    scalar activation (scale folded in), av in transposed layout,
    denominator via ones-matmul of tree-summed exp tiles, then output
    projection per l-block with ri-split PSUM pools so the PSUM WAR
    pipeline never stalls PE. y partials summed on host across groups.
"""

import sys

import numpy as np

try:
    import concourse.bacc as bacc
except ImportError:  # pragma: no cover - fallback for bare environments
    sys.path.insert(0, "/opt/trn_rl_repo")
    import concourse.bacc as bacc

import concourse.mybir as mybir
import concourse.tile as tile
from concourse.bass_utils import run_bass_kernel_spmd

F32 = mybir.dt.float32
BF16 = mybir.dt.bfloat16

# ---- problem constants (hardcoded per contract) ----
B, L, S, C = 2, 2048, 2048, 1024
H, D = 16, 64
SCALE = float(1.0 / np.sqrt(np.float32(D)))
HPC = 4          # heads per core
D2 = 2 * D       # stacked (real|imag) head dim = 128
NCK = C // 128   # contraction chunks = 8
NLB = L // 512   # l-blocks = 4
NSB = S // 512   # s-blocks = 4
NST = S // 128   # s-tiles = 16
NEB = 2          # e-blocks of 512 in C

_CACHE = {}


def _build_program():
    nc = bacc.Bacc("TRN2", target_bir_lowering=False, debug=False, num_devices=8)

    # per-core external inputs (host pre-transposed/cast/stacked)
    xt_r = nc.dram_tensor("xt_r", [C, L], BF16, kind="ExternalInput")
    xt_i = nc.dram_tensor("xt_i", [C, L], BF16, kind="ExternalInput")
    ct_r = nc.dram_tensor("ct_r", [C, S], BF16, kind="ExternalInput")
    ct_i = nc.dram_tensor("ct_i", [C, S], BF16, kind="ExternalInput")
    # wq/wk: [C, HPC, 2, D2]  (c, head, pm, m) ; lhsT tiles
    wq = nc.dram_tensor("wq", [C, HPC, 2, D2], BF16, kind="ExternalInput")
    wk = nc.dram_tensor("wk", [C, HPC, 2, D2], BF16, kind="ExternalInput")
    # wv: [C, 2, HPC*128]  (c, pm, all-head d2) ; rhs tiles
    wv = nc.dram_tensor("wv", [C, 2, HPC * D2], BF16, kind="ExternalInput")
    # wo: [HPC, 128, 2, NEB, 512]  (head, d2row, ri, eblock, e) ; rhs tiles
    wo = nc.dram_tensor("wo", [HPC, D2, 2, NEB, 512], BF16, kind="ExternalInput")

    y_r = nc.dram_tensor("y_r", [L, C], F32, kind="ExternalOutput")
    y_i = nc.dram_tensor("y_i", [L, C], F32, kind="ExternalOutput")

    with tile.TileContext(nc) as tc:
        _emit(nc, tc, xt_r, xt_i, ct_r, ct_i, wq, wk, wv, wo, y_r, y_i)

    nc.compile()
    return nc


def _emit(nc, tc, xt_r, xt_i, ct_r, ct_i, wq, wk, wv, wo, y_r, y_i):
    from contextlib import ExitStack

    ctx = ExitStack()
    with ctx:
        persist = ctx.enter_context(tc.tile_pool(name="persist", bufs=1))

        # persistent attention operands (all bf16)
        qs = persist.tile([128, HPC, L], BF16)            # [d2, h, l]
        ks = persist.tile([128, HPC, S], BF16)            # [d2, h, s]
        vs = persist.tile([128, NST, HPC * D2], BF16)     # [s-part, st, d2all]

        with (
            tc.tile_pool(name="qstr", bufs=2) as q_pool,
            tc.tile_pool(name="wqp", bufs=1) as wq_pool,
            tc.tile_pool(name="ctc", bufs=1) as ctc_pool,
            tc.tile_pool(name="wkv", bufs=1) as wkv_pool,
        ):
            # ---- front-loaded DMA programs ----
            # Activation HWDGE queue: wq halves, xt chunks, wk, wv
            wq_sb = wq_pool.tile([128, NCK, HPC, 2, D2], BF16, tag="wq", name="wq_sb")
            wq_r = wq.rearrange("(ck p) h pm m -> p ck h pm m", p=128)
            for ch in range(2):
                cs = slice(ch * NCK // 2, (ch + 1) * NCK // 2)
                nc.scalar.dma_start(out=wq_sb[:, cs], in_=wq_r[:, cs])
            xtcs = []
            for lb in range(NLB):
                lsl = slice(lb * 512, (lb + 1) * 512)
                xtc = q_pool.tile([128, NCK, 2, 512], BF16, tag="xtc", name="xtc")
                nch = 2 if lb == 0 else 1
                for t, src in ((0, xt_r), (1, xt_i)):
                    for ch in range(nch):
                        cs = slice(ch * NCK // nch, (ch + 1) * NCK // nch)
                        nc.scalar.dma_start(
                            out=xtc[:, cs, t, :],
                            in_=src.rearrange("(ck p) l -> p ck l", p=128)[:, cs, lsl],
                        )
                xtcs.append(xtc)
                if lb == 1:
                    wk_sb = wkv_pool.tile(
                        [128, NCK, HPC, 2, D2], BF16, tag="wk", name="wk_sb"
                    )
                    nc.scalar.dma_start(
                        out=wk_sb, in_=wk.rearrange("(ck p) h pm m -> p ck h pm m", p=128)
                    )
                if lb == 2:
                    wv_sb = wkv_pool.tile(
                        [128, NCK, 2, HPC * D2], BF16, tag="wv", name="wv_sb"
                    )
                    nc.scalar.dma_start(
                        out=wv_sb, in_=wv.rearrange("(ck p) pm n -> p ck pm n", p=128)
                    )
            # SP HWDGE queue: all ct chunks (consumed in phase KV)
            ctcs = []
            for sb in range(NSB):
                ssl = slice(sb * 512, (sb + 1) * 512)
                ctc = ctc_pool.tile([128, NCK, 2, 512], BF16, tag=f"ctc{sb}", name=f"ctc{sb}")
                for t, src in ((0, ct_r), (1, ct_i)):
                    nc.sync.dma_start(
                        out=ctc[:, :, t, :],
                        in_=src.rearrange("(ck p) s -> p ck s", p=128)[:, :, ssl],
                    )
                ctcs.append(ctc)

            # ---------- Phase Q: Q projection from streamed xt chunks ------
            with tc.tile_pool(name="ps_q", bufs=2, space="PSUM") as ps_q:
                for lb in range(NLB):
                    lsl = slice(lb * 512, (lb + 1) * 512)
                    xtc = xtcs[lb]
                    for hp in range(HPC // 2):
                        pq = ps_q.tile([128, 2, 512], F32, tag="pq", name="pq")
                        n = 2 * NCK
                        i = 0
                        for ck in range(NCK):
                            for pm in range(2):
                                for hh in range(2):
                                    nc.tensor.matmul(
                                        pq[:, hh, :],
                                        wq_sb[:, ck, 2 * hp + hh, pm, :],
                                        xtc[:, ck, pm, :],
                                        start=(i == 0),
                                        stop=(i == n - 1),
                                        skip_group_check=True,
                                    )
                                i += 1
                        for hh in range(2):
                            nc.vector.tensor_copy(
                                out=qs[:, 2 * hp + hh, lsl], in_=pq[:, hh, :]
                            )

            # ---------- Phase KV: K + V projections from streamed ct -------
            with (
                tc.tile_pool(name="ps_k", bufs=2, space="PSUM") as ps_k,
                tc.tile_pool(name="ps_v", bufs=2, space="PSUM") as ps_v,
            ):
                for sb in range(NSB):
                    ssl = slice(sb * 512, (sb + 1) * 512)
                    ctc = ctcs[sb]
                    for hp in range(HPC // 2):
                        pk = ps_k.tile([128, 2, 512], F32, tag="pk", name="pk")
                        n = 2 * NCK
                        i = 0
                        for ck in range(NCK):
                            for pm in range(2):
                                for hh in range(2):
                                    nc.tensor.matmul(
                                        pk[:, hh, :],
                                        wk_sb[:, ck, 2 * hp + hh, pm, :],
                                        ctc[:, ck, pm, :],
                                        start=(i == 0),
                                        stop=(i == n - 1),
                                        skip_group_check=True,
                                    )
                                i += 1
                        for hh in range(2):
                            nc.vector.tensor_copy(
                                out=ks[:, 2 * hp + hh, ssl], in_=pk[:, hh, :]
                            )
                    for jt in range(4):
                        st = sb * 4 + jt
                        pv = ps_v.tile([128, 512], F32, tag="pv", name="pv")
                        n = 2 * NCK
                        i = 0
                        for ck in range(NCK):
                            for pm in range(2):
                                nc.tensor.matmul(
                                    pv,
                                    ctc[:, ck, pm, jt * 128:(jt + 1) * 128],
                                    wv_sb[:, ck, pm, :],
                                    start=(i == 0),
                                    stop=(i == n - 1),
                                )
                                i += 1
                        nc.vector.tensor_copy(out=vs[:, st, :], in_=pv)

        # ---------- Phase ATTN: attention + output projection, lb-outer ----
        with (
            tc.tile_pool(name="late", bufs=1) as late_pool,
            tc.tile_pool(name="expp", bufs=4) as exp_pool,
            tc.tile_pool(name="otp", bufs=2) as ot_pool,
            tc.tile_pool(name="ysb", bufs=4) as ysb_pool,
            tc.tile_pool(name="ps_s", bufs=2, space="PSUM") as ps_s,
            tc.tile_pool(name="ps_o", bufs=1, space="PSUM") as ps_o,
            tc.tile_pool(name="ps_d", bufs=1, space="PSUM") as ps_d,
            tc.tile_pool(name="ps_yr", bufs=1, space="PSUM") as ps_yr,
            tc.tile_pool(name="ps_yi", bufs=1, space="PSUM") as ps_yi,
        ):
            ones = late_pool.tile([128, D2], BF16)
            nc.vector.memset(ones, 1.0)
            wo_sb = late_pool.tile([128, HPC, 2, NEB, 512], BF16, tag="wo", name="wo_sb")
            nc.scalar.dma_start(out=wo_sb, in_=wo.rearrange("h p ri eb e -> p h ri eb e"))

            def scores_block(lb, h, expts):
                lsl = slice(lb * 512, (lb + 1) * 512)
                expt = exp_pool.tile([128, NST, 512], BF16, tag="expt", name="expt")
                for pr in range(NST // 2):
                    pscore = ps_s.tile([128, 2, 512], F32, tag="pscore", name="pscore")
                    for j in range(2):
                        st = 2 * pr + j
                        nc.tensor.matmul(
                            pscore[:, j, :],
                            ks[:, h, st * 128:(st + 1) * 128],
                            qs[:, h, lsl],
                            start=True,
                            stop=True,
                            skip_group_check=True,
                        )
                    nc.scalar.activation(
                        out=expt[:, 2 * pr:2 * pr + 2, :],
                        in_=pscore,
                        func=mybir.ActivationFunctionType.Exp,
                        scale=SCALE,
                    )
                expts[(lb, h)] = expt

            def av_block(lb, h, expts, ots):
                expt = expts.pop((lb, h))
                ot = ots[lb]
                pav = ps_o.tile([128, 512], F32, tag="pav", name="pav")
                for st in range(NST):
                    nc.tensor.matmul(
                        pav,
                        vs[:, st, h * D2:(h + 1) * D2],
                        expt[:, st, :],
                        start=(st == 0),
                        stop=(st == NST - 1),
                    )
                # in-place pairwise tree-sum of the 16 s-tiles (WAR after av)
                for step in (1, 2, 4, 8):
                    eng = nc.gpsimd if step == 1 else nc.vector
                    for j in range(0, NST, 2 * step):
                        eng.tensor_add(
                            out=expt[:, j, :], in0=expt[:, j, :], in1=expt[:, j + step, :]
                        )
                pden = ps_d.tile([128, 512], F32, tag="pden", name="pden")
                nc.tensor.matmul(pden, ones, expt[:, 0, :], start=True, stop=True)
                recip = ot_pool.tile([128, 512], F32, tag="recip", name="recip")
                nc.vector.reciprocal(out=recip, in_=pden)
                nc.vector.tensor_mul(out=ot[:, h, :], in0=pav, in1=recip)

            def oproj_block(lb, ots):
                ot = ots.pop(lb)
                for jt in range(4):
                    lt = lb * 4 + jt
                    lrow = slice(lt * 128, (lt + 1) * 128)
                    for eb in range(NEB):
                        esl = slice(eb * 512, (eb + 1) * 512)
                        pys = [
                            ps_yr.tile([128, 512], F32, tag="pyr", name="pyr"),
                            ps_yi.tile([128, 512], F32, tag="pyi", name="pyi"),
                        ]
                        for ri in range(2):
                            for h in range(HPC):
                                nc.tensor.matmul(
                                    pys[ri],
                                    ot[:, h, jt * 128:(jt + 1) * 128],
                                    wo_sb[:, h, ri, eb, :],
                                    start=(h == 0),
                                    stop=(h == HPC - 1),
                                    skip_group_check=True,
                                )
                        yr_t = ysb_pool.tile([128, 512], F32, tag="yrt", name="yrt")
                        nc.vector.tensor_copy(out=yr_t, in_=pys[0])
                        nc.sync.dma_start(out=y_r[lrow, esl], in_=yr_t)
                        yi_t = ysb_pool.tile([128, 512], F32, tag="yit", name="yit")
                        nc.vector.tensor_copy(out=yi_t, in_=pys[1])
                        nc.sync.dma_start(out=y_i[lrow, esl], in_=yi_t)

            # software-pipelined emission: scores stay 2 blocks ahead of av
            expts, ots = {}, {}
            blocks = [(lb, h) for lb in range(NLB) for h in range(HPC)]
            for lb in range(NLB):
                ots[lb] = ot_pool.tile([128, HPC, 512], BF16, tag="ot", name="ot")
            LOOKAHEAD = 2
            for i, (lb, h) in enumerate(blocks):
                if i < LOOKAHEAD:
                    scores_block(lb, h, expts)
                    continue
                scores_block(lb, h, expts)
                av_block(*blocks[i - LOOKAHEAD], expts, ots)
                if blocks[i - LOOKAHEAD][1] == HPC - 1:
                    oproj_block(blocks[i - LOOKAHEAD][0], ots)
            for i in range(len(blocks) - LOOKAHEAD, len(blocks)):
                av_block(*blocks[i], expts, ots)
                if blocks[i][1] == HPC - 1:
                    oproj_block(blocks[i][0], ots)


def _prep_core_inputs(inputs, core):
    """Slice + host-prepare activations/weights for one core."""
    import ml_dtypes

    b = core // 4
    g = core % 4
    hcols = slice(g * HPC * D, (g + 1) * HPC * D)  # 256 channel cols/rows

    wq_r = inputs["wq_r"][:, hcols]
    wq_i = inputs["wq_i"][:, hcols]
    wk_r = inputs["wk_r"][:, hcols]
    wk_i = inputs["wk_i"][:, hcols]
    wv_r = inputs["wv_r"][:, hcols]
    wv_i = inputs["wv_i"][:, hcols]
    wo_r = inputs["wo_r"][hcols, :]
    wo_i = inputs["wo_i"][hcols, :]

    def stack_lhst(wr, wi):
        # [C, HPC, 2, D2]: pm=0 -> [wr | wi], pm=1 -> [-wi | wr]
        out = np.empty((C, HPC, 2, D2), np.float32)
        for hh in range(HPC):
            cs = slice(hh * D, (hh + 1) * D)
            out[:, hh, 0, :D] = wr[:, cs]
            out[:, hh, 0, D:] = wi[:, cs]
            out[:, hh, 1, :D] = -wi[:, cs]
            out[:, hh, 1, D:] = wr[:, cs]
        return out.astype(ml_dtypes.bfloat16)

    def stack_rhs_v(wr, wi):
        # [C, 2, HPC*D2]
        out = np.empty((C, 2, HPC * D2), np.float32)
        for hh in range(HPC):
            cs = slice(hh * D, (hh + 1) * D)
            out[:, 0, hh * D2:hh * D2 + D] = wr[:, cs]
            out[:, 0, hh * D2 + D:(hh + 1) * D2] = wi[:, cs]
            out[:, 1, hh * D2:hh * D2 + D] = -wi[:, cs]
            out[:, 1, hh * D2 + D:(hh + 1) * D2] = wr[:, cs]
        return out.astype(ml_dtypes.bfloat16)

    def stack_wo(wr, wi):
        # [HPC, D2, 2, NEB, 512]; rows 0:D multiply Or, D:D2 multiply Oi
        out = np.empty((HPC, D2, 2, NEB, 512), np.float32)
        for hh in range(HPC):
            rs = slice(hh * D, (hh + 1) * D)
            for eb in range(NEB):
                esl = slice(eb * 512, (eb + 1) * 512)
                out[hh, :D, 0, eb, :] = wr[rs, esl]
                out[hh, D:, 0, eb, :] = -wi[rs, esl]
                out[hh, :D, 1, eb, :] = wi[rs, esl]
                out[hh, D:, 1, eb, :] = wr[rs, esl]
        return out.astype(ml_dtypes.bfloat16)

    bf = ml_dtypes.bfloat16
    return {
        "xt_r": np.ascontiguousarray(inputs["inputs_real"][b].T).astype(bf),
        "xt_i": np.ascontiguousarray(inputs["inputs_imag"][b].T).astype(bf),
        "ct_r": np.ascontiguousarray(inputs["context_real"][b].T).astype(bf),
        "ct_i": np.ascontiguousarray(inputs["context_imag"][b].T).astype(bf),
        "wq": stack_lhst(wq_r, wq_i),
        "wk": stack_lhst(wk_r, wk_i),
        "wv": stack_rhs_v(wv_r, wv_i),
        "wo": stack_wo(wo_r, wo_i),
    }


def get_program():
    if "nc" not in _CACHE:
        _CACHE["nc"] = _build_program()
    return _CACHE["nc"]


def kernel(**inputs):
    nc = get_program()
    in_maps = [_prep_core_inputs(inputs, core) for core in range(8)]
    res = run_bass_kernel_spmd(nc, in_maps, core_ids=list(range(8)))

    yr = np.zeros((B, L, C), np.float32)
    yi = np.zeros((B, L, C), np.float32)
    for core in range(8):
        b = core // 4
        yr[b] += res.results[core]["y_r"]
        yi[b] += res.results[core]["y_i"]
    yr += inputs["bo_r"][None, None, :]
    yi += inputs["bo_i"][None, None, :]
    return np.stack([yr, yi], axis=0)


# revision 11
# speedup vs baseline: 1.4250x; 1.0271x over previous
"""Trainium2 Bass kernel for nn_ComplexCrossAttention.

Sharding: 8 cores = 2 batches x 4 head-groups (4 heads each).

Host prep (free for the HW metric): activations are transposed to [C, L]
and cast to bf16 on the host, so the kernel needs no DMA-xbar transposes
and no fp32->bf16 cast DMAs. Weights are pre-stacked for the complex
matmuls.

Per-core program (phases ordered to eliminate PE stalls):
  Phase Q  (per l-block): stacked complex Q projection from xt chunks
    streamed on the Activation HWDGE queue.
  Phase KV (per s-block): K and V projections sharing ct chunks streamed
    on the SP HWDGE queue (prefetched during Q).
  Phase ATTN (per (l-block, head)): scoresT = (qr.kr+qi.ki), exp via
    scalar activation (scale folded in), av in transposed layout,
    denominator via ones-matmul of tree-summed exp tiles, then output
    projection per l-block with ri-split PSUM pools so the PSUM WAR
    pipeline never stalls PE. y partials summed on host across groups.
"""

import sys

import numpy as np

try:
    import concourse.bacc as bacc
except ImportError:  # pragma: no cover - fallback for bare environments
    sys.path.insert(0, "/opt/trn_rl_repo")
    import concourse.bacc as bacc

import concourse.mybir as mybir
import concourse.tile as tile
from concourse.bass_utils import run_bass_kernel_spmd

F32 = mybir.dt.float32
BF16 = mybir.dt.bfloat16

# ---- problem constants (hardcoded per contract) ----
B, L, S, C = 2, 2048, 2048, 1024
H, D = 16, 64
SCALE = float(1.0 / np.sqrt(np.float32(D)))
HPC = 4          # heads per core
D2 = 2 * D       # stacked (real|imag) head dim = 128
NCK = C // 128   # contraction chunks = 8
NLB = L // 512   # l-blocks = 4
NSB = S // 512   # s-blocks = 4
NST = S // 128   # s-tiles = 16
NEB = 2          # e-blocks of 512 in C

_CACHE = {}


def _build_program():
    nc = bacc.Bacc("TRN2", target_bir_lowering=False, debug=False, num_devices=8)

    # per-core external inputs (host pre-transposed/cast/stacked)
    xt_r = nc.dram_tensor("xt_r", [C, L], BF16, kind="ExternalInput")
    xt_i = nc.dram_tensor("xt_i", [C, L], BF16, kind="ExternalInput")
    ct_r = nc.dram_tensor("ct_r", [C, S], BF16, kind="ExternalInput")
    ct_i = nc.dram_tensor("ct_i", [C, S], BF16, kind="ExternalInput")
    # wq/wk: [C, HPC, 2, D2]  (c, head, pm, m) ; lhsT tiles
    wq = nc.dram_tensor("wq", [C, HPC, 2, D2], BF16, kind="ExternalInput")
    wk = nc.dram_tensor("wk", [C, HPC, 2, D2], BF16, kind="ExternalInput")
    # wv: [C, 2, HPC*128]  (c, pm, all-head d2) ; rhs tiles
    wv = nc.dram_tensor("wv", [C, 2, HPC * D2], BF16, kind="ExternalInput")
    # wo: [HPC, 128, 2, NEB, 512]  (head, d2row, ri, eblock, e) ; rhs tiles
    wo = nc.dram_tensor("wo", [HPC, D2, 2, NEB, 512], BF16, kind="ExternalInput")

    y_r = nc.dram_tensor("y_r", [L, C], F32, kind="ExternalOutput")
    y_i = nc.dram_tensor("y_i", [L, C], F32, kind="ExternalOutput")

    with tile.TileContext(nc) as tc:
        _emit(nc, tc, xt_r, xt_i, ct_r, ct_i, wq, wk, wv, wo, y_r, y_i)

    nc.compile()
    return nc


def _emit(nc, tc, xt_r, xt_i, ct_r, ct_i, wq, wk, wv, wo, y_r, y_i):
    from contextlib import ExitStack

    ctx = ExitStack()
    with ctx:
        persist = ctx.enter_context(tc.tile_pool(name="persist", bufs=1))

        # persistent attention operands (all bf16)
        qs = persist.tile([128, HPC, L], BF16)            # [d2, h, l]
        ks = persist.tile([128, HPC, S], BF16)            # [d2, h, s]
        vs = persist.tile([128, NST, HPC * D2], BF16)     # [s-part, st, d2all]

        # ctc3 + wv outlive the KV phase: V's last s-block is emitted as PE
        # filler inside the attention weave (see below).
        kv_late = ctx.enter_context(tc.tile_pool(name="kv_late", bufs=1))
        with (
            tc.tile_pool(name="qstr", bufs=2) as q_pool,
            tc.tile_pool(name="wqp", bufs=1) as wq_pool,
            tc.tile_pool(name="ctc", bufs=1) as ctc_pool,
            tc.tile_pool(name="wkv", bufs=1) as wkv_pool,
        ):
            # ---- front-loaded DMA programs ----
            # SP HWDGE queue leads with wq (fine-split) so the first Q matmul
            # starts ~2.5us in; Activation queue streams the xt chunks.
            wq_sb = wq_pool.tile([128, NCK, HPC, 2, D2], BF16, tag="wq", name="wq_sb")
            wq_r = wq.rearrange("(ck p) h pm m -> p ck h pm m", p=128)
            for ch in range(4):
                cs = slice(ch * NCK // 4, (ch + 1) * NCK // 4)
                nc.sync.dma_start(out=wq_sb[:, cs], in_=wq_r[:, cs])
            xtcs = []
            for lb in range(NLB):
                lsl = slice(lb * 512, (lb + 1) * 512)
                xtc = q_pool.tile([128, NCK, 2, 512], BF16, tag="xtc", name="xtc")
                nch = 2 if lb == 0 else 1
                for ch in range(nch):
                    cs = slice(ch * NCK // nch, (ch + 1) * NCK // nch)
                    for t, src in ((0, xt_r), (1, xt_i)):
                        nc.scalar.dma_start(
                            out=xtc[:, cs, t, :],
                            in_=src.rearrange("(ck p) l -> p ck l", p=128)[:, cs, lsl],
                        )
                xtcs.append(xtc)
                if lb == 1:
                    wk_sb = wkv_pool.tile(
                        [128, NCK, HPC, 2, D2], BF16, tag="wk", name="wk_sb"
                    )
                    nc.scalar.dma_start(
                        out=wk_sb, in_=wk.rearrange("(ck p) h pm m -> p ck h pm m", p=128)
                    )
                if lb == 2:
                    wv_sb = kv_late.tile(
                        [128, NCK, 2, HPC * D2], BF16, tag="wv", name="wv_sb"
                    )
                    nc.scalar.dma_start(
                        out=wv_sb, in_=wv.rearrange("(ck p) pm n -> p ck pm n", p=128)
                    )
            # SP HWDGE queue: all ct chunks (consumed in phase KV)
            ctcs = []
            for sb in range(NSB):
                ssl = slice(sb * 512, (sb + 1) * 512)
                pool = kv_late if sb == NSB - 1 else ctc_pool
                ctc = pool.tile([128, NCK, 2, 512], BF16, tag=f"ctc{sb}", name=f"ctc{sb}")
                for t, src in ((0, ct_r), (1, ct_i)):
                    nc.sync.dma_start(
                        out=ctc[:, :, t, :],
                        in_=src.rearrange("(ck p) s -> p ck s", p=128)[:, :, ssl],
                    )
                ctcs.append(ctc)

            # ---------- Phase Q: Q projection from streamed xt chunks ------
            with tc.tile_pool(name="ps_q", bufs=2, space="PSUM") as ps_q:
                for lb in range(NLB):
                    lsl = slice(lb * 512, (lb + 1) * 512)
                    xtc = xtcs[lb]
                    for hp in range(HPC // 2):
                        pq = ps_q.tile([128, 2, 512], F32, tag="pq", name="pq")
                        n = 2 * NCK
                        i = 0
                        for ck in range(NCK):
                            for pm in range(2):
                                for hh in range(2):
                                    nc.tensor.matmul(
                                        pq[:, hh, :],
                                        wq_sb[:, ck, 2 * hp + hh, pm, :],
                                        xtc[:, ck, pm, :],
                                        start=(i == 0),
                                        stop=(i == n - 1),
                                        skip_group_check=True,
                                    )
                                i += 1
                        for hh in range(2):
                            nc.vector.tensor_copy(
                                out=qs[:, 2 * hp + hh, lsl], in_=pq[:, hh, :]
                            )

            # ---------- Phase KV: K (all s-blocks), V for s-blocks 0..2 -----
            # V's last s-block is deferred into the attention weave as PE
            # filler for the first two (exp-paced) attention blocks.
            with (
                tc.tile_pool(name="ps_k", bufs=2, space="PSUM") as ps_k,
                tc.tile_pool(name="ps_v", bufs=2, space="PSUM") as ps_v,
            ):
                for sb in range(NSB):
                    ssl = slice(sb * 512, (sb + 1) * 512)
                    ctc = ctcs[sb]
                    for hp in range(HPC // 2):
                        pk = ps_k.tile([128, 2, 512], F32, tag="pk", name="pk")
                        n = 2 * NCK
                        i = 0
                        for ck in range(NCK):
                            for pm in range(2):
                                for hh in range(2):
                                    nc.tensor.matmul(
                                        pk[:, hh, :],
                                        wk_sb[:, ck, 2 * hp + hh, pm, :],
                                        ctc[:, ck, pm, :],
                                        start=(i == 0),
                                        stop=(i == n - 1),
                                        skip_group_check=True,
                                    )
                                i += 1
                        for hh in range(2):
                            nc.vector.tensor_copy(
                                out=ks[:, 2 * hp + hh, ssl], in_=pk[:, hh, :]
                            )
                for sb in range(NSB - 1):
                    ctc = ctcs[sb]
                    for jt in range(4):
                        st = sb * 4 + jt
                        pv = ps_v.tile([128, 512], F32, tag="pv", name="pv")
                        n = 2 * NCK
                        i = 0
                        for ck in range(NCK):
                            for pm in range(2):
                                nc.tensor.matmul(
                                    pv,
                                    ctc[:, ck, pm, jt * 128:(jt + 1) * 128],
                                    wv_sb[:, ck, pm, :],
                                    start=(i == 0),
                                    stop=(i == n - 1),
                                )
                                i += 1
                        nc.vector.tensor_copy(out=vs[:, st, :], in_=pv)

        # ---------- Phase ATTN: attention + output projection, lb-outer ----
        with (
            tc.tile_pool(name="late", bufs=1) as late_pool,
            tc.tile_pool(name="expp", bufs=4) as exp_pool,
            tc.tile_pool(name="otp", bufs=2) as ot_pool,
            tc.tile_pool(name="ysb", bufs=4) as ysb_pool,
            tc.tile_pool(name="ps_s", bufs=2, space="PSUM") as ps_s,
            tc.tile_pool(name="ps_o", bufs=1, space="PSUM") as ps_o,
            tc.tile_pool(name="ps_d", bufs=1, space="PSUM") as ps_d,
            tc.tile_pool(name="ps_yr", bufs=1, space="PSUM") as ps_yr,
            tc.tile_pool(name="ps_yi", bufs=1, space="PSUM") as ps_yi,
        ):
            ones = late_pool.tile([128, D2], BF16)
            nc.vector.memset(ones, 1.0)
            wo_sb = late_pool.tile([128, HPC, 2, NEB, 512], BF16, tag="wo", name="wo_sb")
            nc.scalar.dma_start(out=wo_sb, in_=wo.rearrange("h p ri eb e -> p h ri eb e"))

            # ---- emission helpers: PE work is woven so exp never stalls PE -
            expts, ots = {}, {}
            blocks = [(lb, h) for lb in range(NLB) for h in range(HPC)]
            for lb in range(NLB):
                ots[lb] = ot_pool.tile([128, HPC, 512], BF16, tag="ot", name="ot")

            def emit_score_pair(lb, h, pr):
                lsl = slice(lb * 512, (lb + 1) * 512)
                expt = expts[(lb, h)]
                pscore = ps_s.tile([128, 2, 512], F32, tag="pscore", name="pscore")
                for j in range(2):
                    st = 2 * pr + j
                    nc.tensor.matmul(
                        pscore[:, j, :],
                        ks[:, h, st * 128:(st + 1) * 128],
                        qs[:, h, lsl],
                        start=True,
                        stop=True,
                        skip_group_check=True,
                    )
                nc.scalar.activation(
                    out=expt[:, 2 * pr:2 * pr + 2, :],
                    in_=pscore,
                    func=mybir.ActivationFunctionType.Exp,
                    scale=SCALE,
                )

            def av_chunk_fillers(lb, h):
                """Yield PE filler units for the av + softmax tail of a block."""
                expt = expts[(lb, h)]
                pav = ps_o.tile([128, 512], F32, tag="pav", name="pav")

                def av_chunk(c0):
                    def emit():
                        for st in range(c0, c0 + 4):
                            nc.tensor.matmul(
                                pav,
                                vs[:, st, h * D2:(h + 1) * D2],
                                expt[:, st, :],
                                start=(st == 0),
                                stop=(st == NST - 1),
                                skip_group_check=True,
                            )
                    return emit

                for c0 in range(0, NST, 4):
                    yield av_chunk(c0)

                def tail():
                    # in-place pairwise tree-sum of the 16 s-tiles (WAR after av)
                    del expts[(lb, h)]
                    for step in (1, 2, 4, 8):
                        eng = nc.gpsimd if step == 1 else nc.vector
                        for j in range(0, NST, 2 * step):
                            eng.tensor_add(
                                out=expt[:, j, :], in0=expt[:, j, :],
                                in1=expt[:, j + step, :],
                            )
                    pden = ps_d.tile([128, 512], F32, tag="pden", name="pden")
                    nc.tensor.matmul(
                        pden, ones, expt[:, 0, :], start=True, stop=True,
                        skip_group_check=True,
                    )
                    recip = ot_pool.tile([128, 512], F32, tag="recip", name="recip")
                    nc.vector.reciprocal(out=recip, in_=pden)
                    nc.vector.tensor_mul(out=ot[:, h, :], in0=pav, in1=recip)

                ot = ots[lb]
                yield tail

            def oproj_fillers(lb):
                ot = ots.pop(lb)

                def group(jt, eb):
                    def emit():
                        lt = lb * 4 + jt
                        lrow = slice(lt * 128, (lt + 1) * 128)
                        esl = slice(eb * 512, (eb + 1) * 512)
                        pys = [
                            ps_yr.tile([128, 512], F32, tag="pyr", name="pyr"),
                            ps_yi.tile([128, 512], F32, tag="pyi", name="pyi"),
                        ]
                        for ri in range(2):
                            for h in range(HPC):
                                nc.tensor.matmul(
                                    pys[ri],
                                    ot[:, h, jt * 128:(jt + 1) * 128],
                                    wo_sb[:, h, ri, eb, :],
                                    start=(h == 0),
                                    stop=(h == HPC - 1),
                                    skip_group_check=True,
                                )
                        yr_t = ysb_pool.tile([128, 512], F32, tag="yrt", name="yrt")
                        nc.vector.tensor_copy(out=yr_t, in_=pys[0])
                        nc.sync.dma_start(out=y_r[lrow, esl], in_=yr_t)
                        yi_t = ysb_pool.tile([128, 512], F32, tag="yit", name="yit")
                        nc.vector.tensor_copy(out=yi_t, in_=pys[1])
                        nc.sync.dma_start(out=y_i[lrow, esl], in_=yi_t)
                    return emit

                for jt in range(4):
                    for eb in range(NEB):
                        yield group(jt, eb)

            # Software-pipelined weave: scores run 2 blocks ahead; the av /
            # softmax-tail / output-projection units of older blocks are
            # emitted between score pairs as PE filler so the pscore-bank
            # drain (paced by the Act engine's exp) never idles the PE.
            from collections import deque

            fillers = deque()
            LOOKAHEAD = 2

            def v_filler(jt):
                def emit():
                    st = (NSB - 1) * 4 + jt
                    ctc = ctcs[NSB - 1]
                    pool = ps_yr if jt % 2 == 0 else ps_yi
                    tag = "pyr" if jt % 2 == 0 else "pyi"
                    pv = pool.tile([128, 512], F32, tag=tag, name="pv")
                    n = 2 * NCK
                    i = 0
                    for ck in range(NCK):
                        for pm in range(2):
                            nc.tensor.matmul(
                                pv,
                                ctc[:, ck, pm, jt * 128:(jt + 1) * 128],
                                wv_sb[:, ck, pm, :],
                                start=(i == 0),
                                stop=(i == n - 1),
                                skip_group_check=True,
                            )
                            i += 1
                    nc.vector.tensor_copy(out=vs[:, st, :], in_=pv)
                return emit

            for jt in range(4):
                fillers.append(v_filler(jt))

            def enqueue_block_fillers(i):
                lb, h = blocks[i]
                fillers.extend(av_chunk_fillers(lb, h))
                if h == HPC - 1:
                    fillers.extend(oproj_fillers(lb))

            for i, (lb, h) in enumerate(blocks):
                expts[(lb, h)] = exp_pool.tile(
                    [128, NST, 512], BF16, tag="expt", name="expt"
                )
                if i >= LOOKAHEAD:
                    enqueue_block_fillers(i - LOOKAHEAD)
                for pr in range(NST // 2):
                    emit_score_pair(lb, h, pr)
                    for _ in range(2):
                        if fillers:
                            fillers.popleft()()
            for i in range(len(blocks) - LOOKAHEAD, len(blocks)):
                enqueue_block_fillers(i)
            while fillers:
                fillers.popleft()()


def _prep_core_inputs(inputs, core):
    """Slice + host-prepare activations/weights for one core."""
    import ml_dtypes

    b = core // 4
    g = core % 4
    hcols = slice(g * HPC * D, (g + 1) * HPC * D)  # 256 channel cols/rows

    wq_r = inputs["wq_r"][:, hcols]
    wq_i = inputs["wq_i"][:, hcols]
    wk_r = inputs["wk_r"][:, hcols]
    wk_i = inputs["wk_i"][:, hcols]
    wv_r = inputs["wv_r"][:, hcols]
    wv_i = inputs["wv_i"][:, hcols]
    wo_r = inputs["wo_r"][hcols, :]
    wo_i = inputs["wo_i"][hcols, :]

    def stack_lhst(wr, wi):
        # [C, HPC, 2, D2]: pm=0 -> [wr | wi], pm=1 -> [-wi | wr]
        out = np.empty((C, HPC, 2, D2), np.float32)
        for hh in range(HPC):
            cs = slice(hh * D, (hh + 1) * D)
            out[:, hh, 0, :D] = wr[:, cs]
            out[:, hh, 0, D:] = wi[:, cs]
            out[:, hh, 1, :D] = -wi[:, cs]
            out[:, hh, 1, D:] = wr[:, cs]
        return out.astype(ml_dtypes.bfloat16)

    def stack_rhs_v(wr, wi):
        # [C, 2, HPC*D2]
        out = np.empty((C, 2, HPC * D2), np.float32)
        for hh in range(HPC):
            cs = slice(hh * D, (hh + 1) * D)
            out[:, 0, hh * D2:hh * D2 + D] = wr[:, cs]
            out[:, 0, hh * D2 + D:(hh + 1) * D2] = wi[:, cs]
            out[:, 1, hh * D2:hh * D2 + D] = -wi[:, cs]
            out[:, 1, hh * D2 + D:(hh + 1) * D2] = wr[:, cs]
        return out.astype(ml_dtypes.bfloat16)

    def stack_wo(wr, wi):
        # [HPC, D2, 2, NEB, 512]; rows 0:D multiply Or, D:D2 multiply Oi
        out = np.empty((HPC, D2, 2, NEB, 512), np.float32)
        for hh in range(HPC):
            rs = slice(hh * D, (hh + 1) * D)
            for eb in range(NEB):
                esl = slice(eb * 512, (eb + 1) * 512)
                out[hh, :D, 0, eb, :] = wr[rs, esl]
                out[hh, D:, 0, eb, :] = -wi[rs, esl]
                out[hh, :D, 1, eb, :] = wi[rs, esl]
                out[hh, D:, 1, eb, :] = wr[rs, esl]
        return out.astype(ml_dtypes.bfloat16)

    bf = ml_dtypes.bfloat16
    return {
        "xt_r": np.ascontiguousarray(inputs["inputs_real"][b].T).astype(bf),
        "xt_i": np.ascontiguousarray(inputs["inputs_imag"][b].T).astype(bf),
        "ct_r": np.ascontiguousarray(inputs["context_real"][b].T).astype(bf),
        "ct_i": np.ascontiguousarray(inputs["context_imag"][b].T).astype(bf),
        "wq": stack_lhst(wq_r, wq_i),
        "wk": stack_lhst(wk_r, wk_i),
        "wv": stack_rhs_v(wv_r, wv_i),
        "wo": stack_wo(wo_r, wo_i),
    }


def get_program():
    if "nc" not in _CACHE:
        _CACHE["nc"] = _build_program()
    return _CACHE["nc"]


def kernel(**inputs):
    nc = get_program()
    in_maps = [_prep_core_inputs(inputs, core) for core in range(8)]
    res = run_bass_kernel_spmd(nc, in_maps, core_ids=list(range(8)))

    yr = np.zeros((B, L, C), np.float32)
    yi = np.zeros((B, L, C), np.float32)
    for core in range(8):
        b = core // 4
        yr[b] += res.results[core]["y_r"]
        yi[b] += res.results[core]["y_i"]
    yr += inputs["bo_r"][None, None, :]
    yi += inputs["bo_i"][None, None, :]
    return np.stack([yr, yi], axis=0)
